# revision 3
# baseline (speedup 1.0000x reference)
"""Trainium2 Bass kernel for sorted segment_max (ClusterPool) — v2.

Strategy ("padded transposed stream", no device gather):
  - Host sorts clusters by row count (descending) and deals them round-robin
    to the 8 cores (rank r -> core r%8, position r//8). All cores therefore
    hold clusters of near-identical sizes at each position, so one shared
    chunk schedule (CHUNK_POS positions, each padded to the chunk max count
    L_j) fits every core with ~2% padding.
  - Host builds, per core, a transposed padded bf16 matrix PT [128 feats,
    TOT cols]. bf16 is safe: max() is order-preserving and the 0.4% element
    rounding error is well under the 2e-2 tolerance.
  - Layout per chunk is GROUP-MAJOR: the L=16h slots of each cluster are
    split into 16 slot-groups laid out as [g][cluster][h]. The device then
    reduces with 4 fully-contiguous tensor_tensor(max) folds (measured
    0.54 ns/elem on DVE vs 1.05 for tensor_reduce) + one small
    tensor_reduce over [128, CHUNK_POS, h], all overlapped with a
    full-bandwidth sequential DMA stream (~400 GB/s/core measured).
    Chunks are processed in "valley" order (small first for fast pipeline
    fill, smallest last for a short drain tail).
  - Host scatters core outputs back by rank permutation, upcasts to f32,
    sets empty clusters to -inf.
"""

import os
import sys

import numpy as np

# A previous device fault can leave the cores in a state with ~25% lower
# effective DMA bandwidth; requesting a core reset at NRT init restores it.
os.environ.setdefault("NEURON_RT_RESET_CORES", "1")

sys.path.insert(0, "/opt/trn_rl_repo")

N_POINTS = 2_000_000
D = 128
N_CLUSTERS = 10_000
N_CORES = 8
NPOS = N_CLUSTERS // N_CORES  # positions per core (1250)
CHUNK_POS = 50  # positions per chunk
NCHUNK = NPOS // CHUNK_POS  # 25
NG = 16  # slot groups (fold depth 4)

# 'fold3' : group-major layout, 3 tensor_tensor folds + small tensor_reduce
# 'reduce': flat layout, single tensor_reduce per chunk
MODE = os.environ.get("K2_MODE", "fold3")
# which engine runs the first (largest) fold: 'vector' or 'gpsimd'
F1_ENGINE = os.environ.get("K2_F1", "vector")

_last_results = None


def _apply_drain_patch():
    import concourse.mybir as mybir
    import concourse.tile as tile
    from concourse.vector_clock import ScopedClock

    if getattr(tile.TileContext, "_drain_patched", False):
        return

    def _patched(self, tick_clock, wait_clock):
        nc = self.nc
        nop = nc.sync.nop(nofuse=True, hint="tail_drain_waits")
        wait_clock.add_sem_waits(nop.ins, ScopedClock({None: tick_clock.global_clock}))
        si = nop.ins.sync_info
        waits = list(si.on_wait) if si is not None and si.on_wait else []
        if len(waits) > 1:
            si.on_wait = waits[:1]
            for i in range(1, len(waits)):
                extra = nc.sync.nop(nofuse=True, hint=f"tail_drain_waits_{i}")
                if extra.ins.sync_info is None:
                    extra.ins.sync_info = mybir.SyncInfo(
                        on_wait=waits[i : i + 1], on_update=[]
                    )
                else:
                    extra.ins.sync_info.on_wait = waits[i : i + 1]
        nc.sync.drain()
        nc.all_engine_barrier()
        assert self.sems is not None
        popped = nc._tile_sem_poison_stack.pop()
        assert popped is self._sem_poison
        nc.clear_and_free_semaphores(list(self.sems.allocated().values()))
        nc.all_engine_barrier()

    tile.TileContext._drain_and_barrier = _patched
    tile.TileContext._drain_patched = True


def _build_program(Ls):
    """Shared SPMD program. Ls = per-chunk padded cluster length (len NCHUNK),
    each a multiple of NG."""
    import concourse.bacc as bacc
    import concourse.mybir as mybir
    import concourse.tile as tile

    _apply_drain_patch()

    bf16 = mybir.dt.bfloat16
    TOT = sum(CHUNK_POS * L for L in Ls)
    MAX = mybir.AluOpType.max

    nc = bacc.Bacc(None, num_swdge_queues=2)
    pt_in = nc.dram_tensor("pt", [D, TOT], bf16, kind="ExternalInput")
    o_out = nc.dram_tensor("po", [D, NPOS], bf16, kind="ExternalOutput")

    offs = [0]
    for L in Ls:
        offs.append(offs[-1] + CHUNK_POS * L)
    # valley order: start with small chunks (fast pipeline fill), put the
    # largest in the middle, end with the smallest (short drain tail).
    n = len(Ls)
    order_ = [j for j in range(n - 2, -1, -2)] + [j for j in range((n + 1) % 2, n, 2)]
    with tile.TileContext(nc) as tc:
        with tc.tile_pool(name="p", bufs=1) as pool:
            for j in order_:
                L = Ls[j]
                cols = CHUNK_POS * L
                off = offs[j]
                t = pool.tile([D, cols], bf16, tag="in", bufs=4)
                nc.sync.dma_start(out=t[:], in_=pt_in[:, off : off + cols])
                o = pool.tile([D, CHUNK_POS], bf16, tag="out", bufs=3)
                if MODE == "fold3":
                    h = L // NG
                    prev = t
                    w = cols
                    k = 0
                    while w > CHUNK_POS * h:
                        w //= 2
                        k += 1
                        f = pool.tile([D, w], bf16, tag=f"f{k}", bufs=2)
                        nc.vector.tensor_tensor(
                            out=f[:], in0=prev[:, :w], in1=prev[:, w:], op=MAX
                        )
                        prev = f
                    nc.vector.tensor_reduce(
                        out=o[:],
                        in_=prev[:].rearrange("p (n l) -> p n l", l=h),
                        axis=mybir.AxisListType.X,
                        op=MAX,
                    )
                else:
                    nc.vector.tensor_reduce(
                        out=o[:],
                        in_=t[:].rearrange("p (n l) -> p n l", l=L),
                        axis=mybir.AxisListType.X,
                        op=MAX,
                    )
                nc.gpsimd.dma_start(
                    out=o_out[:, j * CHUNK_POS : (j + 1) * CHUNK_POS], in_=o[:]
                )

    if not nc.is_finalized():
        nc.finalize()
    return nc


def kernel(features, segment_ids, num_clusters):
    global _last_results
    import ml_dtypes
    from concourse.bass_utils import run_bass_kernel_spmd

    bf = ml_dtypes.bfloat16
    features = np.ascontiguousarray(np.asarray(features, dtype=np.float32))
    ids = np.asarray(segment_ids).astype(np.int64)
    nclusters = int(num_clusters)
    assert features.shape == (N_POINTS, D), features.shape
    assert ids.shape == (N_POINTS,)
    assert nclusters == N_CLUSTERS

    # --- host index prep -------------------------------------------------
    ar = np.arange(nclusters)
    gstart = np.searchsorted(ids, ar, side="left")
    gend = np.searchsorted(ids, ar + 1, side="left")
    gcounts = gend - gstart

    order = np.argsort(-gcounts, kind="stable")  # ranks by descending count
    oc = gcounts[order]  # counts in rank order
    os_ = gstart[order]  # starts in rank order

    # chunk schedule: chunk j covers ranks [8*CHUNK_POS*j, 8*CHUNK_POS*(j+1))
    Ls = []
    for j in range(NCHUNK):
        L = int(oc[8 * CHUNK_POS * j])  # max count in chunk (sorted desc)
        L = max(NG, (L + NG - 1) // NG * NG)  # pad to multiple of NG
        Ls.append(L)
    TOT = sum(CHUNK_POS * L for L in Ls)

    featbf = features.astype(bf)
    pts = []
    for c in range(N_CORES):
        colidx = np.empty(TOT, dtype=np.int64)
        off = 0
        for j, L in enumerate(Ls):
            p0 = CHUNK_POS * j
            rr = 8 * (p0 + np.arange(CHUNK_POS)) + c  # ranks of this chunk
            s = os_[rr]
            cnt = np.maximum(oc[rr], 1)
            if MODE == "fold3":
                h = L // NG
                # slot index for layout position [g, p, si] is g*h + si
                slot = (
                    np.arange(NG)[:, None, None] * h
                    + np.arange(h)[None, None, :]
                )  # [NG, 1, h]
                rows = s[None, :, None] + np.minimum(
                    slot, (cnt - 1)[None, :, None]
                )  # [NG, CHUNK_POS, h]
            else:
                sl = np.arange(L)
                rows = s[:, None] + np.minimum(sl[None, :], (cnt - 1)[:, None])
            colidx[off : off + CHUNK_POS * L] = rows.reshape(-1)
            off += CHUNK_POS * L
        np.clip(colidx, 0, N_POINTS - 1, out=colidx)
        pts.append(np.ascontiguousarray(featbf[colidx].T))

    # --- build + run ------------------------------------------------------
    nc = _build_program(Ls)
    in_maps = [{"pt": pts[c]} for c in range(N_CORES)]
    res = run_bass_kernel_spmd(nc, in_maps, list(range(N_CORES)))
    _last_results = res

    # --- host combine -----------------------------------------------------
    full = np.empty((nclusters, D), dtype=np.float32)
    for c in range(N_CORES):
        po = np.asarray(res.results[c]["po"]).astype(np.float32)  # [128, NPOS]
        rr = 8 * np.arange(NPOS) + c
        full[order[rr]] = po.T
    full[gcounts == 0] = -np.inf
    return full
